# revision 33
# baseline (speedup 1.0000x reference)
"""CTBG circuit kernel for Trainium2, data-parallel over batch on 8 NeuronCores.

Network (per reference):
  gpe_out = x @ (gpe_w * gpe_mask.T) + gpe_b              [B, 1536]
  gpi_in  = concat([x, gpe_out], -1)                      [B, 3072]
  gpi_out = gpi_in @ (gpi_w * gpi_mask.T) + gpi_b         [B, 1536]
  h1 = relu(gpi_out @ w1 + b1); h2 = relu(h1 @ w2 + b2)
  out = relu(h2 @ w3 + b3)                                [B, 6]

Key algebraic identity: gpe_out and gpi_out feed forward with no
intervening nonlinearity, so the masked front end folds into one
[1536, 512] weight computed ON DEVICE once per launch:

  mw_gpe = gpe_w * gpe_mask.T
  mw_gpi = gpi_w * gpi_mask.T
  M      = mw_gpi[1536:] @ w1                       [1536, 512]
  Wfold  = mw_gpi[:1536] @ w1 + mw_gpe @ M          [1536, 512]
  bfold  = gpe_b @ M + gpi_b @ w1 + b1              [512]
  h1 = relu(x @ Wfold + bfold)   -> h2 -> out       (per batch row)

The fold itself is SHARDED across the 8 cores: core c computes rows
[c*192, (c+1)*192) of M (then of Wfold), which takes only the
corresponding COLUMN slices of the masks/weights (sliced host-side, a
pure layout op) — so each core streams ~5 MB of fold operands instead
of ~28 MB, and does 1/8 of the fold matmuls. Slices are assembled with
two DRAM AllGathers (M, then Wfold).

Host prep is layout/dtype only (no FLOPs): bf16 casts, transposes of
x/gpe_w/gpi_w, and column slicing.

Per-core phases (BS = 2048 batch rows):
  F0:  stream sliced mask columns + w^T columns, DVE-multiply in place.
  F1s: M_slice = sum_v mwgpiT[v, uslice]^T @ w1[v]  -> DRAM, AllGather.
  F2s: Wf_slice = sum_v mwgpiT[v, islice]^T w1[v]
                + sum_u mwgpeT[u, islice]^T M[u]    -> DRAM, AllGather.
  bias fold: tiny matmuls on gathered M + PE transpose of [1, 512] row.
  B:   per 512-row tile: h1 = relu(Wfold^T x^T), h2, out -> [6, BS] f32;
       host transposes + concats.
"""

import numpy as np
import ml_dtypes

BF = ml_dtypes.bfloat16

NCORES = 8
B = 16384
BS = B // NCORES          # 2048 rows per core
BT = 512                  # batch tile (matmul free dim)
NBT = BS // BT            # 4
D1 = 1536                 # gpe input dim (x features)
D3 = 3072                 # gpi input dim
H = 512                   # mlp hidden
A = 6                     # action dim
SL = D1 // NCORES         # 192: fold rows per core

NI = D1 // 128            # 12 i-chunks (x features)
NU = D1 // 128            # 12 u-chunks (gpe outputs)
NV = D1 // 128            # 12 v-chunks (gpi outputs)
NH = H // 128             # 4 h-chunks (mlp hidden)

_CACHE = {}


def _build():
    import concourse.bacc as bacc
    import concourse.tile as tile
    from concourse import mybir
    from concourse.masks import make_identity

    FP32 = mybir.dt.float32
    BF16 = mybir.dt.bfloat16
    Act = mybir.ActivationFunctionType

    nc = bacc.Bacc(None, num_devices=NCORES)

    xT_d = nc.dram_tensor("xT", [D1, BS], BF16, kind="ExternalInput")
    # column slices for this core's fold rows: gpi gets [islice | uslice]
    # (384 cols), gpe gets [islice] (192 cols)
    gpims_d = nc.dram_tensor("gpims", [D1, 2 * SL], BF16, kind="ExternalInput")
    gpiwTs_d = nc.dram_tensor("gpiwTs", [D1, 2 * SL], BF16, kind="ExternalInput")
    gpems_d = nc.dram_tensor("gpems", [D1, SL], BF16, kind="ExternalInput")
    gpewTs_d = nc.dram_tensor("gpewTs", [D1, SL], BF16, kind="ExternalInput")
    w1_d = nc.dram_tensor("w1", [D1, H], BF16, kind="ExternalInput")
    w2_d = nc.dram_tensor("w2", [H, H], BF16, kind="ExternalInput")
    w3_d = nc.dram_tensor("w3", [H, A], BF16, kind="ExternalInput")
    gpeb_d = nc.dram_tensor("gpe_b", [D1], FP32, kind="ExternalInput")
    gpib_d = nc.dram_tensor("gpi_b", [D1], FP32, kind="ExternalInput")
    b1_d = nc.dram_tensor("b1", [H], FP32, kind="ExternalInput")
    b2_d = nc.dram_tensor("b2", [H], FP32, kind="ExternalInput")
    b3_d = nc.dram_tensor("b3", [A], FP32, kind="ExternalInput")
    o_d = nc.dram_tensor("out", [A, BS], FP32, kind="ExternalOutput")

    RG = [list(range(NCORES))]

    with tile.TileContext(nc) as tc:
        with (
            tc.tile_pool(name="wp", bufs=1) as wp,           # persistent
            tc.tile_pool(name="tp", bufs=2) as tp,           # wT transients
            tc.tile_pool(name="xp", bufs=3) as xp,           # x tiles
            tc.tile_pool(name="ap", bufs=1) as ap,           # activations
            tc.tile_pool(name="dp", bufs=1, space="DRAM") as dp,
            tc.tile_pool(name="psp", bufs=3, space="PSUM") as psp,
            tc.tile_pool(name="ps2", bufs=1, space="PSUM") as ps2p,
            tc.tile_pool(name="pso", bufs=2, space="PSUM") as psop,
            tc.tile_pool(name="pst", bufs=1, space="PSUM") as pstp,
        ):
            # ---- w1 first (gates F1s), then sliced gpi, gpe
            w1t = []
            for v in range(NV):
                t = wp.tile([128, H], BF16, tag=f"w1_{v}")
                nc.sync.dma_start(out=t[:, :], in_=w1_d[v * 128:(v + 1) * 128, :])
                w1t.append(t)

            # masked gpi columns, [v-part, 384]: cols 0:192 = islice,
            # 192:384 = uslice
            mwgpi = []
            for v in range(NV):
                m = wp.tile([128, 2 * SL], BF16, tag=f"mwgpi{v}")
                nc.sync.dma_start(out=m[:, :], in_=gpims_d[v * 128:(v + 1) * 128, :])
                wt = tp.tile([128, 2 * SL], BF16, tag="gwT")
                nc.gpsimd.dma_start(out=wt[:, :],
                                    in_=gpiwTs_d[v * 128:(v + 1) * 128, :])
                nc.vector.tensor_mul(m[:, :], m[:, :], wt[:, :])
                mwgpi.append(m)

            # masked gpe columns, [u-part, 192]: cols = islice
            mwgpe = []
            for u in range(NU):
                m = wp.tile([128, SL], BF16, tag=f"mwgpe{u}")
                nc.sync.dma_start(out=m[:, :], in_=gpems_d[u * 128:(u + 1) * 128, :])
                wt = tp.tile([128, SL], BF16, tag="ewT")
                nc.gpsimd.dma_start(out=wt[:, :],
                                    in_=gpewTs_d[u * 128:(u + 1) * 128, :])
                nc.vector.tensor_mul(m[:, :], m[:, :], wt[:, :])
                mwgpe.append(m)

            # ---- small stuff: w2, w3, biases
            w2t = []
            for k in range(NH):
                t = wp.tile([128, H], BF16, tag=f"w2_{k}")
                nc.sync.dma_start(out=t[:, :], in_=w2_d[k * 128:(k + 1) * 128, :])
                w2t.append(t)
            w3t = []
            for k in range(NH):
                t = wp.tile([128, A], BF16, tag=f"w3_{k}")
                nc.sync.dma_start(out=t[:, :], in_=w3_d[k * 128:(k + 1) * 128, :])
                w3t.append(t)

            ident = wp.tile([128, 128], FP32, tag="ident")
            make_identity(nc, ident[:, :])

            def load_bias_cols(b_dram, n, tag):
                nat = wp.tile([n, 128], FP32, tag=f"{tag}_nat")
                nc.sync.dma_start(out=nat[:, :],
                                  in_=b_dram.rearrange("(c p) -> c p", p=128))
                ps = pstp.tile([128, n], FP32, tag="pst")
                nc.tensor.transpose(ps[:, :], nat[:, :], ident[0:n, 0:n])
                sb = wp.tile([128, n], FP32, tag=tag)
                nc.vector.tensor_copy(sb[:, :], ps[:, :])
                return sb

            gpeb_sb = load_bias_cols(gpeb_d, NU, "gpeb")
            gpib_sb = load_bias_cols(gpib_d, NV, "gpib")
            b2_sb = load_bias_cols(b2_d, NH, "b2sb")
            gpeb_bf = wp.tile([128, NU], BF16, tag="gpebf")
            nc.vector.tensor_copy(gpeb_bf[:, :], gpeb_sb[:, :])
            gpib_bf = wp.tile([128, NV], BF16, tag="gpibf")
            nc.vector.tensor_copy(gpib_bf[:, :], gpib_sb[:, :])
            b1row = wp.tile([1, H], FP32, tag="b1row")
            nc.sync.dma_start(out=b1row[:, :],
                              in_=b1_d.rearrange("(one h) -> one h", one=1))
            b3_sb = wp.tile([A, 1], FP32, tag="b3sb")
            nc.sync.dma_start(out=b3_sb[:, :],
                              in_=b3_d.rearrange("(a one) -> a one", one=1))

            # ---- x tiles stream in the background
            xt = [[None] * NI for _ in range(NBT)]
            for t_i in range(NBT):
                for i in range(NI):
                    t = xp.tile([128, BT], BF16, tag=f"xt{i}")
                    q = nc.gpsimd if (i % 2) else nc.sync
                    q.dma_start(out=t[:, :],
                                in_=xT_d[i * 128:(i + 1) * 128,
                                         t_i * BT:(t_i + 1) * BT])
                    xt[t_i][i] = t

            # ---- F1s: M_slice[r, h] = sum_v mwgpiT[v, 1536+uslice][r] w1[v]
            # slice rows split as 128 + 64
            msl_dram = dp.tile([SL, H], BF16, tag="msl_d")
            for g, (r0, rn) in enumerate([(0, 128), (128, SL - 128)]):
                ps = psp.tile([128, H], FP32, tag="ps")
                for v in range(NV):
                    nc.tensor.matmul(ps[0:rn, :],
                                     mwgpi[v][:, SL + r0:SL + r0 + rn],
                                     w1t[v][:, :],
                                     start=(v == 0), stop=(v == NV - 1))
                sb = wp.tile([128, H], BF16, tag=f"mslice{g}")
                nc.scalar.activation(sb[0:rn, :], ps[0:rn, :], Act.Copy)
                nc.sync.dma_start(out=msl_dram[r0:r0 + rn, :], in_=sb[0:rn, :])
            mfull_dram = dp.tile([D1, H], BF16, tag="mfull_d")
            nc.gpsimd.collective_compute(
                "AllGather", mybir.AluOpType.bypass, replica_groups=RG,
                ins=[msl_dram[:, :].opt()], outs=[mfull_dram[:, :].opt()])
            Mt = []
            for u in range(NU):
                t = wp.tile([128, H], BF16, tag=f"M{u}")
                q = nc.gpsimd if (u % 2) else nc.sync
                q.dma_start(out=t[:, :], in_=mfull_dram[u * 128:(u + 1) * 128, :])
                Mt.append(t)

            # ---- F2s: Wf_slice = gpi-x-part + mwgpe-slice^T @ M
            wfs_dram = dp.tile([SL, H], BF16, tag="wfs_d")
            for g, (r0, rn) in enumerate([(0, 128), (128, SL - 128)]):
                ps = psp.tile([128, H], FP32, tag="ps")
                for v in range(NV):
                    nc.tensor.matmul(ps[0:rn, :],
                                     mwgpi[v][:, r0:r0 + rn],
                                     w1t[v][:, :],
                                     start=(v == 0), stop=False)
                for u in range(NU):
                    nc.tensor.matmul(ps[0:rn, :],
                                     mwgpe[u][:, r0:r0 + rn],
                                     Mt[u][:, :],
                                     start=False, stop=(u == NU - 1))
                sb = wp.tile([128, H], BF16, tag=f"wfslice{g}")
                nc.scalar.activation(sb[0:rn, :], ps[0:rn, :], Act.Copy)
                nc.sync.dma_start(out=wfs_dram[r0:r0 + rn, :], in_=sb[0:rn, :])
            wff_dram = dp.tile([D1, H], BF16, tag="wff_d")
            nc.gpsimd.collective_compute(
                "AllGather", mybir.AluOpType.bypass, replica_groups=RG,
                ins=[wfs_dram[:, :].opt()], outs=[wff_dram[:, :].opt()])
            Wf = []
            for i in range(NI):
                t = wp.tile([128, H], BF16, tag=f"Wf{i}")
                q = nc.gpsimd if (i % 2) else nc.sync
                q.dma_start(out=t[:, :], in_=wff_dram[i * 128:(i + 1) * 128, :])
                Wf.append(t)

            # ---- bias fold: bfold = gpe_b @ M + gpi_b @ w1 + b1 -> [128, 4]
            psb = ps2p.tile([1, H], FP32, tag="psb")
            for v in range(NV):
                nc.tensor.matmul(psb[:, :], gpib_bf[:, v:v + 1], w1t[v][:, :],
                                 start=(v == 0), stop=False)
            for u in range(NU):
                nc.tensor.matmul(psb[:, :], gpeb_bf[:, u:u + 1], Mt[u][:, :],
                                 start=False, stop=(u == NU - 1))
            brow = wp.tile([1, H], FP32, tag="brow")
            nc.vector.tensor_add(brow[:, :], psb[:, :], b1row[:, :])
            bfold = wp.tile([128, NH], FP32, tag="bfold")
            for c in range(NH):
                ps = pstp.tile([128, 1], FP32, tag="pstc")
                nc.tensor.transpose(ps[:, :], brow[0:1, c * 128:(c + 1) * 128],
                                    ident[0:1, 0:1])
                nc.scalar.activation(bfold[:, c:c + 1], ps[:, :], Act.Copy)

            # ---- B: batch pass over 4 tiles of 512 rows
            for t_i in range(NBT):
                h1 = []
                for hc in range(NH):
                    ps = psp.tile([128, BT], FP32, tag="ps")
                    for i in range(NI):
                        nc.tensor.matmul(ps[:, :],
                                         Wf[i][:, hc * 128:(hc + 1) * 128],
                                         xt[t_i][i][:, :],
                                         start=(i == 0), stop=(i == NI - 1))
                    h = ap.tile([128, BT], BF16, tag=f"h1_{hc}")
                    nc.scalar.activation(h[:, :], ps[:, :], Act.Relu,
                                         bias=bfold[:, hc:hc + 1])
                    h1.append(h)

                h2 = []
                for mc in range(NH):
                    ps = psp.tile([128, BT], FP32, tag="ps")
                    for k in range(NH):
                        nc.tensor.matmul(ps[:, :],
                                         w2t[k][:, mc * 128:(mc + 1) * 128],
                                         h1[k][:, :],
                                         start=(k == 0), stop=(k == NH - 1))
                    h = ap.tile([128, BT], BF16, tag=f"h2_{mc}")
                    nc.scalar.activation(h[:, :], ps[:, :], Act.Relu,
                                         bias=b2_sb[:, mc:mc + 1])
                    h2.append(h)

                pso = psop.tile([A, BT], FP32, tag="pso")
                for k in range(NH):
                    nc.tensor.matmul(pso[:, :], w3t[k][:, :], h2[k][:, :],
                                     start=(k == 0), stop=(k == NH - 1))
                osb = ap.tile([A, BT], FP32, tag="osb")
                nc.scalar.activation(osb[:, :], pso[:, :], Act.Relu,
                                     bias=b3_sb[:, 0:1])
                nc.sync.dma_start(out=o_d[:, t_i * BT:(t_i + 1) * BT],
                                  in_=osb[:, :])

    nc.finalize()
    return nc


def _get_nc():
    if "nc" not in _CACHE:
        _CACHE["nc"] = _build()
    return _CACHE["nc"]


def _prep_inputs(inputs):
    """Host-side layout/dtype prep only (no network FLOPs): bf16 casts,
    transposes, and per-core column slicing of the fold operands."""
    f = {k: np.asarray(v) for k, v in inputs.items()}
    xT = np.ascontiguousarray(f["x"].astype(BF).T)            # [1536, B]
    gpem = f["gpe_mask"].astype(BF)                           # [u, i]
    gpewT = np.ascontiguousarray(f["gpe_w"].astype(BF).T)     # [u, i]
    gpim = f["gpi_mask"].astype(BF)                           # [v, j]
    gpiwT = np.ascontiguousarray(f["gpi_w"].astype(BF).T)     # [v, j]
    shared = {
        "w1": np.ascontiguousarray(f["w1"].astype(BF)),
        "w2": np.ascontiguousarray(f["w2"].astype(BF)),
        "w3": np.ascontiguousarray(f["w3"].astype(BF)),
        "gpe_b": np.ascontiguousarray(f["gpe_b"], dtype=np.float32),
        "gpi_b": np.ascontiguousarray(f["gpi_b"], dtype=np.float32),
        "b1": np.ascontiguousarray(f["b1"], dtype=np.float32),
        "b2": np.ascontiguousarray(f["b2"], dtype=np.float32),
        "b3": np.ascontiguousarray(f["b3"], dtype=np.float32),
    }
    in_maps = []
    for c in range(NCORES):
        isl = slice(c * SL, (c + 1) * SL)
        usl = slice(D1 + c * SL, D1 + (c + 1) * SL)
        in_maps.append(dict(
            shared,
            xT=np.ascontiguousarray(xT[:, c * BS:(c + 1) * BS]),
            gpims=np.ascontiguousarray(
                np.concatenate([gpim[:, isl], gpim[:, usl]], axis=1)),
            gpiwTs=np.ascontiguousarray(
                np.concatenate([gpiwT[:, isl], gpiwT[:, usl]], axis=1)),
            gpems=np.ascontiguousarray(gpem[:, isl]),
            gpewTs=np.ascontiguousarray(gpewT[:, isl]),
        ))
    return in_maps


def _run(inputs, trace=False):
    from concourse.bass_utils import run_bass_kernel_spmd

    nc = _get_nc()
    in_maps = _prep_inputs(inputs)
    res = run_bass_kernel_spmd(nc, in_maps, list(range(NCORES)), trace=trace)
    out = np.concatenate(
        [np.asarray(res.results[c]["out"]).T for c in range(NCORES)], axis=0)
    return out.astype(np.float32), res


def kernel(**inputs):
    out, _ = _run(inputs, trace=False)
    return out


# revision 34
# speedup vs baseline: 1.0180x; 1.0180x over previous
"""CTBG circuit kernel for Trainium2, data-parallel over batch on 8 NeuronCores.

Network (per reference):
  gpe_out = x @ (gpe_w * gpe_mask.T) + gpe_b              [B, 1536]
  gpi_in  = concat([x, gpe_out], -1)                      [B, 3072]
  gpi_out = gpi_in @ (gpi_w * gpi_mask.T) + gpi_b         [B, 1536]
  h1 = relu(gpi_out @ w1 + b1); h2 = relu(h1 @ w2 + b2)
  out = relu(h2 @ w3 + b3)                                [B, 6]

Key algebraic identity: gpe_out and gpi_out feed forward with no
intervening nonlinearity, so the masked front end folds into one
[1536, 512] weight computed ON DEVICE once per launch:

  mw_gpe = gpe_w * gpe_mask.T
  mw_gpi = gpi_w * gpi_mask.T
  M      = mw_gpi[1536:] @ w1                       [1536, 512]
  Wfold  = mw_gpi[:1536] @ w1 + mw_gpe @ M          [1536, 512]
  bfold  = gpe_b @ M + gpi_b @ w1 + b1              [512]
  h1 = relu(x @ Wfold + bfold)   -> h2 -> out       (per batch row)

The fold itself is SHARDED across the 8 cores: core c computes rows
[c*192, (c+1)*192) of M (then of Wfold), which takes only the
corresponding COLUMN slices of the masks/weights (sliced host-side, a
pure layout op) — so each core streams ~5 MB of fold operands instead
of ~28 MB, and does 1/8 of the fold matmuls. Slices are assembled with
two DRAM AllGathers (M, then Wfold).

Host prep is layout/dtype only (no FLOPs): bf16 casts, transposes of
x/gpe_w/gpi_w, and column slicing.

Per-core phases (BS = 2048 batch rows):
  F0:  stream sliced mask columns + w^T columns, DVE-multiply in place.
  F1s: M_slice = sum_v mwgpiT[v, uslice]^T @ w1[v]  -> DRAM, AllGather.
  F2s: Wf_slice = sum_v mwgpiT[v, islice]^T w1[v]
                + sum_u mwgpeT[u, islice]^T M[u]    -> DRAM, AllGather.
  bias fold: tiny matmuls on gathered M + PE transpose of [1, 512] row.
  B:   per 512-row tile: h1 = relu(Wfold^T x^T), h2, out -> [6, BS] f32;
       host transposes + concats.
"""

import numpy as np
import ml_dtypes

BF = ml_dtypes.bfloat16

NCORES = 8
B = 16384
BS = B // NCORES          # 2048 rows per core
BT = 512                  # batch tile (matmul free dim)
NBT = BS // BT            # 4
D1 = 1536                 # gpe input dim (x features)
D3 = 3072                 # gpi input dim
H = 512                   # mlp hidden
A = 6                     # action dim
SL = D1 // NCORES         # 192: fold rows per core

NI = D1 // 128            # 12 i-chunks (x features)
NU = D1 // 128            # 12 u-chunks (gpe outputs)
NV = D1 // 128            # 12 v-chunks (gpi outputs)
NH = H // 128             # 4 h-chunks (mlp hidden)

_CACHE = {}


def _build():
    import concourse.bacc as bacc
    import concourse.tile as tile
    from concourse import mybir
    from concourse.masks import make_identity

    FP32 = mybir.dt.float32
    BF16 = mybir.dt.bfloat16
    Act = mybir.ActivationFunctionType

    nc = bacc.Bacc(None, num_devices=NCORES)

    xT_d = nc.dram_tensor("xT", [D1, BS], BF16, kind="ExternalInput")
    # column slices for this core's fold rows: gpi gets [islice | uslice]
    # (384 cols), gpe gets [islice] (192 cols)
    gpims_d = nc.dram_tensor("gpims", [D1, 2 * SL], BF16, kind="ExternalInput")
    gpiwTs_d = nc.dram_tensor("gpiwTs", [D1, 2 * SL], BF16, kind="ExternalInput")
    gpems_d = nc.dram_tensor("gpems", [D1, SL], BF16, kind="ExternalInput")
    gpewTs_d = nc.dram_tensor("gpewTs", [D1, SL], BF16, kind="ExternalInput")
    w1_d = nc.dram_tensor("w1", [D1, H], BF16, kind="ExternalInput")
    w2_d = nc.dram_tensor("w2", [H, H], BF16, kind="ExternalInput")
    w3_d = nc.dram_tensor("w3", [H, A], BF16, kind="ExternalInput")
    gpeb_d = nc.dram_tensor("gpe_b", [D1], FP32, kind="ExternalInput")
    gpib_d = nc.dram_tensor("gpi_b", [D1], FP32, kind="ExternalInput")
    b1_d = nc.dram_tensor("b1", [H], FP32, kind="ExternalInput")
    b2_d = nc.dram_tensor("b2", [H], FP32, kind="ExternalInput")
    b3_d = nc.dram_tensor("b3", [A], FP32, kind="ExternalInput")
    o_d = nc.dram_tensor("out", [A, BS], FP32, kind="ExternalOutput")

    RG = [list(range(NCORES))]

    with tile.TileContext(nc) as tc:
        with (
            tc.tile_pool(name="wp", bufs=1) as wp,           # persistent
            tc.tile_pool(name="tp", bufs=2) as tp,           # wT transients
            tc.tile_pool(name="xp", bufs=3) as xp,           # x tiles
            tc.tile_pool(name="ap", bufs=1) as ap,           # activations
            tc.tile_pool(name="dp", bufs=1, space="DRAM") as dp,
            tc.tile_pool(name="psp", bufs=3, space="PSUM") as psp,
            tc.tile_pool(name="ps2", bufs=1, space="PSUM") as ps2p,
            tc.tile_pool(name="pso", bufs=2, space="PSUM") as psop,
            tc.tile_pool(name="pst", bufs=1, space="PSUM") as pstp,
        ):
            # ---- w1 first (gates F1s), then sliced gpi, gpe
            w1t = []
            for v in range(NV):
                t = wp.tile([128, H], BF16, tag=f"w1_{v}")
                nc.sync.dma_start(out=t[:, :], in_=w1_d[v * 128:(v + 1) * 128, :])
                w1t.append(t)

            # masked gpi columns, [v-part, 384]: cols 0:192 = islice,
            # 192:384 = uslice
            mwgpi = []
            for v in range(NV):
                m = wp.tile([128, 2 * SL], BF16, tag=f"mwgpi{v}")
                nc.sync.dma_start(out=m[:, :], in_=gpims_d[v * 128:(v + 1) * 128, :])
                wt = tp.tile([128, 2 * SL], BF16, tag="gwT")
                nc.gpsimd.dma_start(out=wt[:, :],
                                    in_=gpiwTs_d[v * 128:(v + 1) * 128, :])
                nc.vector.tensor_mul(m[:, :], m[:, :], wt[:, :])
                mwgpi.append(m)

            # masked gpe columns, [u-part, 192]: cols = islice
            mwgpe = []
            for u in range(NU):
                m = wp.tile([128, SL], BF16, tag=f"mwgpe{u}")
                nc.sync.dma_start(out=m[:, :], in_=gpems_d[u * 128:(u + 1) * 128, :])
                wt = tp.tile([128, SL], BF16, tag="ewT")
                nc.gpsimd.dma_start(out=wt[:, :],
                                    in_=gpewTs_d[u * 128:(u + 1) * 128, :])
                nc.vector.tensor_mul(m[:, :], m[:, :], wt[:, :])
                mwgpe.append(m)

            # ---- small stuff: w2, w3, biases
            w2t = []
            for k in range(NH):
                t = wp.tile([128, H], BF16, tag=f"w2_{k}")
                nc.sync.dma_start(out=t[:, :], in_=w2_d[k * 128:(k + 1) * 128, :])
                w2t.append(t)
            w3t = []
            for k in range(NH):
                t = wp.tile([128, A], BF16, tag=f"w3_{k}")
                nc.sync.dma_start(out=t[:, :], in_=w3_d[k * 128:(k + 1) * 128, :])
                w3t.append(t)

            ident = wp.tile([128, 128], FP32, tag="ident")
            make_identity(nc, ident[:, :])

            def load_bias_cols(b_dram, n, tag):
                nat = wp.tile([n, 128], FP32, tag=f"{tag}_nat")
                nc.sync.dma_start(out=nat[:, :],
                                  in_=b_dram.rearrange("(c p) -> c p", p=128))
                ps = pstp.tile([128, n], FP32, tag="pst")
                nc.tensor.transpose(ps[:, :], nat[:, :], ident[0:n, 0:n])
                sb = wp.tile([128, n], FP32, tag=tag)
                nc.vector.tensor_copy(sb[:, :], ps[:, :])
                return sb

            gpeb_sb = load_bias_cols(gpeb_d, NU, "gpeb")
            gpib_sb = load_bias_cols(gpib_d, NV, "gpib")
            b2_sb = load_bias_cols(b2_d, NH, "b2sb")
            gpeb_bf = wp.tile([128, NU], BF16, tag="gpebf")
            nc.vector.tensor_copy(gpeb_bf[:, :], gpeb_sb[:, :])
            gpib_bf = wp.tile([128, NV], BF16, tag="gpibf")
            nc.vector.tensor_copy(gpib_bf[:, :], gpib_sb[:, :])
            b1row = wp.tile([1, H], FP32, tag="b1row")
            nc.sync.dma_start(out=b1row[:, :],
                              in_=b1_d.rearrange("(one h) -> one h", one=1))
            b3_sb = wp.tile([A, 1], FP32, tag="b3sb")
            nc.sync.dma_start(out=b3_sb[:, :],
                              in_=b3_d.rearrange("(a one) -> a one", one=1))

            # ---- x tiles stream in the background
            xt = [[None] * NI for _ in range(NBT)]
            for t_i in range(NBT):
                for i in range(NI):
                    t = xp.tile([128, BT], BF16, tag=f"xt{i}")
                    q = nc.gpsimd if (i % 2) else nc.sync
                    q.dma_start(out=t[:, :],
                                in_=xT_d[i * 128:(i + 1) * 128,
                                         t_i * BT:(t_i + 1) * BT])
                    xt[t_i][i] = t

            # ---- F1s: M_slice[r, h] = sum_v mwgpiT[v, 1536+uslice][r] w1[v]
            # slice rows split as 128 + 64
            msl_dram = dp.tile([SL, H], BF16, tag="msl_d")
            for g, (r0, rn) in enumerate([(0, 128), (128, SL - 128)]):
                ps = psp.tile([128, H], FP32, tag="ps")
                for v in range(NV):
                    nc.tensor.matmul(ps[0:rn, :],
                                     mwgpi[v][:, SL + r0:SL + r0 + rn],
                                     w1t[v][:, :],
                                     start=(v == 0), stop=(v == NV - 1))
                sb = wp.tile([128, H], BF16, tag=f"mslice{g}")
                nc.scalar.activation(sb[0:rn, :], ps[0:rn, :], Act.Copy)
                nc.sync.dma_start(out=msl_dram[r0:r0 + rn, :], in_=sb[0:rn, :])
            mfull_dram = dp.tile([D1, H], BF16, tag="mfull_d")
            nc.gpsimd.collective_compute(
                "AllGather", mybir.AluOpType.bypass, replica_groups=RG,
                ins=[msl_dram[:, :].opt()], outs=[mfull_dram[:, :].opt()])
            Mt = []
            for u in range(NU):
                t = wp.tile([128, H], BF16, tag=f"M{u}")
                nc.sync.dma_start(out=t[:, :],
                                  in_=mfull_dram[u * 128:(u + 1) * 128, :])
                Mt.append(t)

            # ---- F2s: Wf_slice = gpi-x-part + mwgpe-slice^T @ M
            wfs_dram = dp.tile([SL, H], BF16, tag="wfs_d")
            for g, (r0, rn) in enumerate([(0, 128), (128, SL - 128)]):
                ps = psp.tile([128, H], FP32, tag="ps")
                for v in range(NV):
                    nc.tensor.matmul(ps[0:rn, :],
                                     mwgpi[v][:, r0:r0 + rn],
                                     w1t[v][:, :],
                                     start=(v == 0), stop=False)
                for u in range(NU):
                    nc.tensor.matmul(ps[0:rn, :],
                                     mwgpe[u][:, r0:r0 + rn],
                                     Mt[u][:, :],
                                     start=False, stop=(u == NU - 1))
                sb = wp.tile([128, H], BF16, tag=f"wfslice{g}")
                nc.scalar.activation(sb[0:rn, :], ps[0:rn, :], Act.Copy)
                nc.sync.dma_start(out=wfs_dram[r0:r0 + rn, :], in_=sb[0:rn, :])
            wff_dram = dp.tile([D1, H], BF16, tag="wff_d")
            nc.gpsimd.collective_compute(
                "AllGather", mybir.AluOpType.bypass, replica_groups=RG,
                ins=[wfs_dram[:, :].opt()], outs=[wff_dram[:, :].opt()])
            Wf = []
            for i in range(NI):
                t = wp.tile([128, H], BF16, tag=f"Wf{i}")
                nc.sync.dma_start(out=t[:, :],
                                  in_=wff_dram[i * 128:(i + 1) * 128, :])
                Wf.append(t)

            # ---- bias fold: bfold = gpe_b @ M + gpi_b @ w1 + b1 -> [128, 4]
            psb = ps2p.tile([1, H], FP32, tag="psb")
            for v in range(NV):
                nc.tensor.matmul(psb[:, :], gpib_bf[:, v:v + 1], w1t[v][:, :],
                                 start=(v == 0), stop=False)
            for u in range(NU):
                nc.tensor.matmul(psb[:, :], gpeb_bf[:, u:u + 1], Mt[u][:, :],
                                 start=False, stop=(u == NU - 1))
            brow = wp.tile([1, H], FP32, tag="brow")
            nc.vector.tensor_add(brow[:, :], psb[:, :], b1row[:, :])
            bfold = wp.tile([128, NH], FP32, tag="bfold")
            for c in range(NH):
                ps = pstp.tile([128, 1], FP32, tag="pstc")
                nc.tensor.transpose(ps[:, :], brow[0:1, c * 128:(c + 1) * 128],
                                    ident[0:1, 0:1])
                nc.scalar.activation(bfold[:, c:c + 1], ps[:, :], Act.Copy)

            # ---- B: batch pass over 4 tiles of 512 rows
            for t_i in range(NBT):
                h1 = []
                for hc in range(NH):
                    ps = psp.tile([128, BT], FP32, tag="ps")
                    for i in range(NI):
                        nc.tensor.matmul(ps[:, :],
                                         Wf[i][:, hc * 128:(hc + 1) * 128],
                                         xt[t_i][i][:, :],
                                         start=(i == 0), stop=(i == NI - 1))
                    h = ap.tile([128, BT], BF16, tag=f"h1_{hc}")
                    nc.scalar.activation(h[:, :], ps[:, :], Act.Relu,
                                         bias=bfold[:, hc:hc + 1])
                    h1.append(h)

                h2 = []
                for mc in range(NH):
                    ps = psp.tile([128, BT], FP32, tag="ps")
                    for k in range(NH):
                        nc.tensor.matmul(ps[:, :],
                                         w2t[k][:, mc * 128:(mc + 1) * 128],
                                         h1[k][:, :],
                                         start=(k == 0), stop=(k == NH - 1))
                    h = ap.tile([128, BT], BF16, tag=f"h2_{mc}")
                    nc.scalar.activation(h[:, :], ps[:, :], Act.Relu,
                                         bias=b2_sb[:, mc:mc + 1])
                    h2.append(h)

                pso = psop.tile([A, BT], FP32, tag="pso")
                for k in range(NH):
                    nc.tensor.matmul(pso[:, :], w3t[k][:, :], h2[k][:, :],
                                     start=(k == 0), stop=(k == NH - 1))
                osb = ap.tile([A, BT], FP32, tag="osb")
                nc.scalar.activation(osb[:, :], pso[:, :], Act.Relu,
                                     bias=b3_sb[:, 0:1])
                nc.sync.dma_start(out=o_d[:, t_i * BT:(t_i + 1) * BT],
                                  in_=osb[:, :])

    nc.finalize()
    return nc


def _get_nc():
    if "nc" not in _CACHE:
        _CACHE["nc"] = _build()
    return _CACHE["nc"]


def _prep_inputs(inputs):
    """Host-side layout/dtype prep only (no network FLOPs): bf16 casts,
    transposes, and per-core column slicing of the fold operands."""
    f = {k: np.asarray(v) for k, v in inputs.items()}
    xT = np.ascontiguousarray(f["x"].astype(BF).T)            # [1536, B]
    gpem = f["gpe_mask"].astype(BF)                           # [u, i]
    gpewT = np.ascontiguousarray(f["gpe_w"].astype(BF).T)     # [u, i]
    gpim = f["gpi_mask"].astype(BF)                           # [v, j]
    gpiwT = np.ascontiguousarray(f["gpi_w"].astype(BF).T)     # [v, j]
    shared = {
        "w1": np.ascontiguousarray(f["w1"].astype(BF)),
        "w2": np.ascontiguousarray(f["w2"].astype(BF)),
        "w3": np.ascontiguousarray(f["w3"].astype(BF)),
        "gpe_b": np.ascontiguousarray(f["gpe_b"], dtype=np.float32),
        "gpi_b": np.ascontiguousarray(f["gpi_b"], dtype=np.float32),
        "b1": np.ascontiguousarray(f["b1"], dtype=np.float32),
        "b2": np.ascontiguousarray(f["b2"], dtype=np.float32),
        "b3": np.ascontiguousarray(f["b3"], dtype=np.float32),
    }
    in_maps = []
    for c in range(NCORES):
        isl = slice(c * SL, (c + 1) * SL)
        usl = slice(D1 + c * SL, D1 + (c + 1) * SL)
        in_maps.append(dict(
            shared,
            xT=np.ascontiguousarray(xT[:, c * BS:(c + 1) * BS]),
            gpims=np.ascontiguousarray(
                np.concatenate([gpim[:, isl], gpim[:, usl]], axis=1)),
            gpiwTs=np.ascontiguousarray(
                np.concatenate([gpiwT[:, isl], gpiwT[:, usl]], axis=1)),
            gpems=np.ascontiguousarray(gpem[:, isl]),
            gpewTs=np.ascontiguousarray(gpewT[:, isl]),
        ))
    return in_maps


def _run(inputs, trace=False):
    from concourse.bass_utils import run_bass_kernel_spmd

    nc = _get_nc()
    in_maps = _prep_inputs(inputs)
    res = run_bass_kernel_spmd(nc, in_maps, list(range(NCORES)), trace=trace)
    out = np.concatenate(
        [np.asarray(res.results[c]["out"]).T for c in range(NCORES)], axis=0)
    return out.astype(np.float32), res


def kernel(**inputs):
    out, _ = _run(inputs, trace=False)
    return out
